# revision 22
# baseline (speedup 1.0000x reference)
"""Causal self-attention (T=4096, C=2048, 16 heads) on 8 TRN2 NeuronCores.

Sharding: tensor-parallel over heads (2 heads/core) for QKV + attention,
then per-head AllToAlls redistribute the attention output to
token-parallel (512 tokens/core) for the output projection.

This version merges the QKV projection (phase 1) and attention (phase 2)
into one interleaved instruction stream: query-chunk j's score/exp/PV
pipeline runs while token-chunk j+1's QKV matmuls keep the tensor engine
dense, so the scalar engine's exp throughput (the phase-2 binding
resource) hides entirely under phase-1 matmuls and chunk-tail latency
never idles the PE. Softmax denominators are computed on the PE via a
ones-matmul (broadcast partition reduce); gpsimd runs ONLY the two
AllToAlls, so no chunk tail can queue behind an in-flight collective.
Attention-output DMA loads are issued after all a2a_in writes so an
in-flight collective never blocks the sync queue. Phase 3 splits
even/odd source tiles so the head-0 half overlaps the head-1 AllToAll.
"""
import sys
import types

sys.path.insert(0, "/opt/trn_rl_repo")

import ml_dtypes
import numpy as np

from concourse import bacc, tile
import concourse.mybir as mybir
from concourse.bass_utils import run_bass_kernel_spmd

F32 = mybir.dt.float32
BF16 = mybir.dt.bfloat16
FP16 = mybir.dt.float16
NP_BF16 = np.dtype(ml_dtypes.bfloat16)

T, C = 4096, 2048
H, D = 16, 128
W = 8                  # cores
HL = H // W            # heads per core (2)
CL = HL * D            # local attention-output columns (256)
KT = C // 128          # contraction tiles (16)
TC1 = 512              # token chunk (phase 1 == phase 2 chunking)
NC1 = T // TC1         # 8
TC2 = 512
NC2 = T // TC2         # 8
TL = T // W            # tokens per core for the projection (512)
SCALE = float(1.0 / np.sqrt(D))

TRACE = False          # test harness sets kernel.TRACE = True for profiling
LAST_RESULT = {}       # test harness reads exec_time_ns from here

_cache = {}


def _build():
    nc = bacc.Bacc("TRN2", target_bir_lowering=False, debug=False, num_devices=W)
    xT_d = nc.dram_tensor("xT", [C, T], BF16, kind="ExternalInput")
    wqkT_d = nc.dram_tensor("wqkT", [C, 2 * CL], BF16, kind="ExternalInput")
    wvT_d = nc.dram_tensor("wvT", [C, CL], BF16, kind="ExternalInput")
    wpT_d = nc.dram_tensor("wpT", [C, C], BF16, kind="ExternalInput")
    out_d = nc.dram_tensor("out", [TL, C], F32, kind="ExternalOutput")

    with tile.TileContext(nc) as tc:
        with tc.tile_pool(name="res", bufs=1) as res, \
             tc.tile_pool(name="dram", bufs=1, space="DRAM") as dram:
            # per-(head, token-half) A2A buffers (bf16, normalized): shard
            # j = token chunk j. Four 512KB collectives instead of two 1MB
            # ones: each phase-3 quarter starts as soon as its half lands,
            # pipelining the PE against the remaining transfers.
            a2a_in = [[dram.tile([W, 128, TC2 // 2], BF16,
                                 tag=f"a2a_in{h}_{hf}",
                                 name=f"a2a_in{h}_{hf}") for hf in range(2)]
                      for h in range(HL)]
            a2a_out = [[dram.tile([W, 128, TC2 // 2], BF16,
                                  tag=f"a2a_out{h}_{hf}",
                                  name=f"a2a_out{h}_{hf}") for hf in range(2)]
                       for h in range(HL)]

            # resident q/k (transposed, [d, t]) and V ([s, d]), fp16
            qT = [res.tile([128, T], FP16, tag=f"qT{h}", name=f"qT{h}")
                  for h in range(HL)]
            kT = [res.tile([128, T], FP16, tag=f"kT{h}", name=f"kT{h}")
                  for h in range(HL)]
            V = [res.tile([128, CL], FP16, tag=f"V{i}", name=f"V{i}")
                 for i in range(T // 128)]

            # all-ones [128, 128] fp16: lhsT for the denominator matmul
            # (ones.T @ esum broadcasts the partition sum to all partitions)
            ones32 = res.tile([128, 128], F32, tag="ones32", name="ones32")
            nc.gpsimd.memset(ones32[:], 1.0)
            ones = res.tile([128, 128], FP16, tag="ones", name="ones")
            nc.vector.tensor_copy(ones[:], ones32[:])

            # 4 diagonal causal masks (keep where t >= s within the tile):
            # mask dk applies to s-tile k = 4j + dk of query chunk j
            masks = []
            for dk in range(4):
                m32 = res.tile([128, TC2], F32, tag="m32", name=f"m32_{dk}")
                nc.gpsimd.memset(m32[:], 1.0)
                mb = res.tile([128, TC2], FP16, tag=f"mask{dk}",
                              name=f"mask{dk}")
                nc.vector.tensor_copy(mb[:], m32[:])
                nc.gpsimd.affine_select(
                    out=mb[:], in_=mb[:],
                    compare_op=mybir.AluOpType.is_ge,
                    fill=0.0,
                    base=-128 * dk,
                    channel_multiplier=-1,
                    pattern=[[1, TC2]],
                )
                masks.append(mb)

            # projection weight, resident for phase 3 (prefetched mid-stream)
            wp = [[None] * KT for _ in range(C // 512)]

            # ---------------- merged phases 1+2 ----------------
            with tc.tile_pool(name="wpool", bufs=1) as wpool, \
                 tc.tile_pool(name="xpool", bufs=2) as xpool, \
                 tc.tile_pool(name="ph2", bufs=4) as p2, \
                 tc.tile_pool(name="es", bufs=2) as es, \
                 tc.tile_pool(name="dn", bufs=2) as dn, \
                 tc.tile_pool(name="a2s", bufs=3) as a2s, \
                 tc.tile_pool(name="pqp", bufs=1, space="PSUM") as pqp, \
                 tc.tile_pool(name="pvp", bufs=1, space="PSUM") as pvp, \
                 tc.tile_pool(name="ps2s", bufs=2, space="PSUM") as ps2s, \
                 tc.tile_pool(name="ps2o", bufs=1, space="PSUM") as ps2o, \
                 tc.tile_pool(name="psd", bufs=1, space="PSUM") as psd:
                wqk = [[None] * 4 for _ in range(KT)]
                wv = [None] * KT

                # zero-init the 4 e buffers: the diagonal mask multiplies
                # read the full half-tile, and 0 x stale-NaN (uninitialized
                # SBUF) would poison esum on each buffer's first use
                for i_ in range(4):
                    et = p2.tile([128, 2 * TC2], FP16, tag="e",
                                 name=f"einit{i_}")
                    nc.gpsimd.memset(et[:], 0.0)

                def load_x(j):
                    ts = []
                    for k in range(KT):
                        t_ = xpool.tile([128, TC1], BF16, tag=f"x{k}",
                                        name=f"x{j}_{k}")
                        nc.sync.dma_start(
                            t_[:],
                            xT_d.ap()[k * 128:(k + 1) * 128,
                                      j * TC1:(j + 1) * TC1],
                        )
                        ts.append(t_)
                    return ts

                # initial DMA: x chunk 0 interleaved with wqk column 0 + wv
                # (everything the m=0 group needs), then wqk columns 1-3
                xt0 = []
                for k in range(KT):
                    t_ = xpool.tile([128, TC1], BF16, tag=f"x{k}",
                                    name=f"x0_{k}")
                    nc.sync.dma_start(
                        t_[:], xT_d.ap()[k * 128:(k + 1) * 128, 0:TC1])
                    xt0.append(t_)
                    t2 = wpool.tile([128, 128], BF16, tag=f"wqk{k}_0",
                                    name=f"wqk{k}_0")
                    nc.sync.dma_start(
                        t2[:], wqkT_d.ap()[k * 128:(k + 1) * 128, 0:128])
                    wqk[k][0] = t2
                    t3 = wpool.tile([128, CL], BF16, tag=f"wv{k}",
                                    name=f"wv{k}")
                    nc.sync.dma_start(
                        t3[:], wvT_d.ap()[k * 128:(k + 1) * 128, :])
                    wv[k] = t3
                for m in range(1, 4):
                    for k in range(KT):
                        t2 = wpool.tile([128, 128], BF16, tag=f"wqk{k}_{m}",
                                        name=f"wqk{k}_{m}")
                        nc.sync.dma_start(
                            t2[:],
                            wqkT_d.ap()[k * 128:(k + 1) * 128,
                                        m * 128:(m + 1) * 128])
                        wqk[k][m] = t2

                # ---- phase-2 pair pipeline over (head, chunk, pair) ----
                pair_seq = []
                for j in range(NC2):
                    for h in range(HL):
                        for p in range(2 * (j + 1)):
                            pair_seq.append((h, j, p))
                NPAIRS = len(pair_seq)  # 144

                cstate = {}

                def chunk_state(h, j):
                    key = (h, j)
                    if key not in cstate:
                        cstate[key] = {
                            "po": ps2o.tile([128, TC2], F32, tag="po",
                                            name=f"po_{h}_{j}"),
                            "esum": es.tile([128, TC2], FP16, tag="esum",
                                            name=f"esum_{h}_{j}"),
                        }
                    return cstate[key]

                def diag_off(j, k):
                    # diagonal s-tile k only scores queries t >= 128*dk
                    dk = k - 4 * j
                    return 128 * dk if dk > 0 else 0

                def emit_pair_scores(h, j, p):
                    ps = ps2s.tile([128, 2 * TC2], F32, tag="ps",
                                   name=f"ps_{h}_{j}_{p}")
                    for half in range(2):
                        k = 2 * p + half
                        off = diag_off(j, k)
                        nc.tensor.matmul(
                            ps[:, half * TC2 + off:(half + 1) * TC2],
                            kT[h][:, k * 128:(k + 1) * 128],
                            qT[h][:, j * TC2 + off:(j + 1) * TC2],
                            start=True, stop=True)
                    return ps

                emitted = {}
                # cap: scores for pair idx >= cap may not be emitted yet —
                # their qT/kT source chunk is still being evacuated. Raised
                # per m-group as the evacs are emitted.
                cur = {"emit": 0, "cons": 0, "cap": 4}
                LA = 2

                def consume(idx):
                    h, j, p = pair_seq[idx]
                    nk = (j + 1) * 4
                    st = chunk_state(h, j)
                    ps = emitted.pop(idx)
                    e = p2.tile([128, 2 * TC2], FP16, tag="e", name=f"e{idx}")
                    if p >= 2 * j:
                        # diagonal pair: the score matmuls only wrote
                        # [off:] of each half — exp of the unwritten PSUM
                        # region is stale garbage (observed NaN on HW), so
                        # narrow the activation to the written range and
                        # zero the causally-dead columns explicitly (on DVE,
                        # NOT gpsimd: a gpsimd memset in the tail would
                        # queue behind the blocking A2A instruction)
                        for half in range(2):
                            k = 2 * p + half
                            off = diag_off(j, k)
                            if off > 0:
                                nc.vector.memset(
                                    e[:, half * TC2:half * TC2 + off], 0.0)
                            nc.scalar.activation(
                                e[:, half * TC2 + off:(half + 1) * TC2],
                                ps[:, half * TC2 + off:(half + 1) * TC2],
                                mybir.ActivationFunctionType.Exp,
                                scale=SCALE)
                    else:
                        # one exp per pair amortizes ACT init cost
                        nc.scalar.activation(
                            e[:], ps[:],
                            mybir.ActivationFunctionType.Exp,
                            scale=SCALE)
                    for half in range(2):
                        k = 2 * p + half
                        dk = k - 4 * j
                        if dk >= 0:
                            eh = e[:, half * TC2:(half + 1) * TC2]
                            nc.vector.tensor_mul(eh, eh, masks[dk][:])
                    for half in range(2):
                        k = 2 * p + half
                        eh = e[:, half * TC2:(half + 1) * TC2]
                        if k == 0:
                            nc.vector.tensor_copy(st["esum"][:], eh)
                        else:
                            nc.vector.tensor_add(st["esum"][:],
                                                 st["esum"][:], eh)
                    # emit the lookahead scores only AFTER this pair's exp is
                    # on the ACT queue: the new ps tile reuses the buffer of
                    # pair idx-LA+... the oldest live pair, and the WAR dep on
                    # its exp read only exists once that exp is emitted
                    ni = idx + LA
                    if ni < min(NPAIRS, cur["cap"]) and cur["emit"] <= ni:
                        emitted[ni] = emit_pair_scores(*pair_seq[ni])
                        cur["emit"] = ni + 1
                    for half in range(2):
                        # exp of a narrowed score tile leaves stale data left
                        # of `off`; the mask zeroed it for esum, and P@V
                        # skips those columns (causally zero for this s-tile)
                        k = 2 * p + half
                        off = diag_off(j, k)
                        nc.tensor.matmul(
                            st["po"][:, off:],
                            V[k][:, h * 128:(h + 1) * 128],
                            e[:, half * TC2 + off:(half + 1) * TC2],
                            start=(k == 0), stop=(k == nk - 1))
                    if 2 * p + 2 != nk:
                        return
                    # chunk tail: denominator via ones-matmul on the PE
                    # (broadcast partition reduce), reciprocal on DVE, then
                    # normalize po during PSUM evacuation
                    den = psd.tile([128, TC2], F32, tag="den",
                                   name=f"den_{h}_{j}")
                    nc.tensor.matmul(den[:], ones[:], st["esum"][:],
                                     start=True, stop=True)
                    # evacuate to SBUF first: reciprocal_approx_fast is a
                    # custom DVE op and reading PSUM directly produced NaNs
                    # on hardware
                    dsb = dn.tile([128, TC2], F32, tag="dsb",
                                  name=f"dsb_{h}_{j}")
                    nc.scalar.copy(dsb[:], den[:])
                    rec = dn.tile([128, TC2], F32, tag="rec",
                                  name=f"rec_{h}_{j}")
                    nc.vector.reciprocal_approx_fast(out=rec[:], in_=dsb[:])
                    att = a2s.tile([128, TC2], BF16, tag="att",
                                   name=f"att_{h}_{j}")
                    nc.vector.tensor_mul(att[:], st["po"][:], rec[:])
                    for hf in range(2):
                        nc.sync.dma_start(
                            a2a_in[h][hf][j, :, :],
                            att[:, hf * (TC2 // 2):(hf + 1) * (TC2 // 2)])
                    del cstate[(h, j)]
                    if j == NC2 - 1:
                        # fire this head's A2As the moment its data is ready;
                        # gpsimd carries nothing else, so the blocking
                        # collective instruction stalls no other work
                        for hf in range(2):
                            nc.gpsimd.collective_compute(
                                "AllToAll",
                                mybir.AluOpType.bypass,
                                ins=[a2a_in[h][hf].opt()],
                                outs=[a2a_out[h][hf].opt()],
                                replica_groups=[list(range(W))],
                            )

                def pump(n):
                    for _ in range(n):
                        ci = cur["cons"]
                        if ci >= NPAIRS:
                            return
                        while cur["emit"] < min(ci + LA, NPAIRS,
                                                cur["cap"]):
                            ei = cur["emit"]
                            emitted[ei] = emit_pair_scores(*pair_seq[ei])
                            cur["emit"] += 1
                        assert cur["emit"] > ci, (ci, cur)
                        consume(ci)
                        cur["cons"] += 1

                def p1_group(xt, j, m, slots):
                    # one QKV m-group (32 MMs) split into 4 sub-blocks with
                    # phase-2 pairs pumped between them so ACT never lags
                    pq = pqp.tile([128, TC1], F32, tag="pq",
                                  name=f"pq{j}_{m}")
                    pv = pvp.tile([128, CL], F32, tag="pv", name=f"pv{j}_{m}")
                    for kb in range(0, KT, 4):
                        for k in range(kb, kb + 4):
                            nc.tensor.matmul(pq[:], wqk[k][m][:], xt[k][:],
                                             start=(k == 0),
                                             stop=(k == KT - 1))
                            nc.tensor.matmul(
                                pv[:],
                                xt[k][:, m * 128:(m + 1) * 128],
                                wv[k][:],
                                start=(k == 0), stop=(k == KT - 1))
                        pump(slots.pop(0) if slots else 0)
                    dest = qT[m] if m < HL else kT[m - HL]
                    nc.vector.tensor_copy(dest[:, j * TC1:(j + 1) * TC1],
                                          pq[:])
                    nc.scalar.copy(V[j * 4 + m][:], pv[:])

                def quota_slots(quota, nslots):
                    base = quota // nslots
                    slots = [base] * nslots
                    for z in range(quota - base * nslots):
                        slots[z] += 1
                    return slots

                # P1 chunk 0: no pairs ready yet
                for m in range(4):
                    p1_group(xt0, 0, m, [])
                # P1 chunks 1..7: pump the pairs of query chunk jj-1
                for jj in range(1, NC1):
                    xt = load_x(jj)
                    if jj == 2:
                        # prefetch the projection weight behind x chunk 2
                        for oc in range(C // 512):
                            for kc in range(KT):
                                t_ = res.tile([128, 512], BF16,
                                              tag=f"wp{oc}_{kc}",
                                              name=f"wp{oc}_{kc}")
                                nc.sync.dma_start(
                                    t_[:],
                                    wpT_d.ap()[kc * 128:(kc + 1) * 128,
                                               oc * 512:(oc + 1) * 512],
                                )
                                wp[oc][kc] = t_
                    if jj < NC1 - 1:
                        slots = quota_slots(4 * jj, 17)
                        # cap: next-chunk (h0,jj) lookahead scores need qT0
                        # chunk jj — evacuated at the end of m=0's group
                        caps = [2 * jj * (jj + 1)] + [NPAIRS] * 4
                    else:
                        # extended final window: chunk-6 pairs of both heads
                        # (28) plus the first 14 pairs of (h0, 7)
                        slots = [2] * 8 + [3] * 8 + [2]
                        # pair idx needs: 112..125 qT0(7) [m=0 evac],
                        # 126..127 kT0(7) [m=2], 128..141 qT1(7) [m=1],
                        # 142..143 kT1(7) [m=3]; caps[m] applies DURING
                        # group m, i.e. before that group's evac is emitted
                        caps = [112, 126, 126, 142, NPAIRS]
                    for m in range(4):
                        cur["cap"] = caps[m]
                        p1_group(xt, jj, m, slots[4 * m:4 * (m + 1)])
                    cur["cap"] = caps[4]
                    pump(sum(slots[16:]))
                # tail: (h0,7) p14-15 -> A2A h0 fires, then (h1,7)'s 16
                # pairs cover the h0 transfer, then A2A h1 (hidden under
                # phase-3's even half)
                pump(NPAIRS)

            # ---------------- phase 3: output projection ----------------
            with tc.tile_pool(name="p3a", bufs=1) as p3a, \
                 tc.tile_pool(name="acc3", bufs=1) as acc3, \
                 tc.tile_pool(name="p3o", bufs=3) as p3o, \
                 tc.tile_pool(name="ps3", bufs=2, space="PSUM") as ps3:
                attn = {}

                def load_attn(h, hf):
                    # issued from the scalar (ACT) hwdge queue: idle at the
                    # tail, and a load waiting on an in-flight collective
                    # must not block att writes queued on sync behind it
                    for i in range(W):
                        kc = i * HL + h
                        t_ = p3a.tile([128, TL // 2], BF16,
                                      tag=f"at{kc}_{hf}",
                                      name=f"at{kc}_{hf}")
                        nc.scalar.dma_start(t_[:], a2a_out[h][hf][i, :, :])
                        attn[(kc, hf)] = t_

                for h in range(HL):
                    for hf in range(2):
                        load_attn(h, hf)

                # even-kc (head-0 sourced) per token-half first — each
                # quarter starts as soon as its 512KB A2A lands, so the PE
                # pipelines against the remaining transfers; odd halves then
                # finish in psum and DVE adds the f32 SBUF spill.
                accs = {}
                for hf in range(2):
                    for oc in range(4):
                        for tt in (2 * hf, 2 * hf + 1):
                            p3 = ps3.tile([128, 512], F32, tag="p3",
                                          name=f"p3e_{oc}_{tt}")
                            for kc in range(0, KT, 2):
                                nc.tensor.matmul(
                                    p3[:],
                                    attn[(kc, hf)][:, (tt - 2 * hf) * 128:
                                                   (tt - 2 * hf + 1) * 128],
                                    wp[oc][kc][:],
                                    start=(kc == 0), stop=(kc == KT - 2))
                            # spill on DVE: the scalar queue holds the attn
                            # load descriptors, which block on collective
                            # semaphores — a scalar copy behind them would
                            # stall the psum rotation
                            acc = acc3.tile([128, 512], F32,
                                            tag=f"acc{oc}_{tt}",
                                            name=f"acc{oc}_{tt}")
                            nc.vector.tensor_copy(acc[:], p3[:])
                            accs[(oc, tt)] = acc
                for hf in range(2):
                    for oc in range(4):
                        for tt in (2 * hf, 2 * hf + 1):
                            p3 = ps3.tile([128, 512], F32, tag="p3",
                                          name=f"p3o_{oc}_{tt}")
                            for kc in range(1, KT, 2):
                                nc.tensor.matmul(
                                    p3[:],
                                    attn[(kc, hf)][:, (tt - 2 * hf) * 128:
                                                   (tt - 2 * hf + 1) * 128],
                                    wp[oc][kc][:],
                                    start=(kc == 1), stop=(kc == KT - 1))
                            ob = p3o.tile([128, 512], F32, tag="ob")
                            nc.vector.tensor_add(ob[:], accs[(oc, tt)][:],
                                                 p3[:])
                            nc.sync.dma_start(
                                out_d.ap()[tt * 128:(tt + 1) * 128,
                                           oc * 512:(oc + 1) * 512],
                                ob[:])

    nc.compile()
    return nc


def _maybe_install_trace_hook():
    try:
        import antenv
        from trn_agent_boot.trn_boot import _ntff_profile_via_ctypes
        hook = _ntff_profile_via_ctypes("/opt/axon/libaxon_pjrt.so")
        mod = types.ModuleType("antenv.axon_hooks")
        mod.get_axon_ntff_profile_hook = lambda: hook
        mod.set_axon_ntff_profile_hook = lambda h: None
        sys.modules["antenv.axon_hooks"] = mod
        antenv.axon_hooks = mod
        return True
    except Exception:
        return False


def kernel(x, w_attn, w_proj):
    x = np.ascontiguousarray(x, dtype=np.float32)
    w_attn = np.ascontiguousarray(w_attn, dtype=np.float32)
    w_proj = np.ascontiguousarray(w_proj, dtype=np.float32)

    if "nc" not in _cache:
        _cache["nc"] = _build()
    nc = _cache["nc"]

    xT = np.ascontiguousarray(x.T).astype(NP_BF16)
    wpT = np.ascontiguousarray(w_proj.T).astype(NP_BF16)
    in_maps = []
    for c in range(W):
        r0 = CL * c
        wqk = np.concatenate(
            [w_attn[r0:r0 + CL], w_attn[C + r0:C + r0 + CL]], axis=0)
        wqkT = np.ascontiguousarray(wqk.T).astype(NP_BF16)
        wvT = np.ascontiguousarray(
            w_attn[2 * C + r0:2 * C + r0 + CL].T).astype(NP_BF16)
        in_maps.append({"xT": xT, "wqkT": wqkT, "wvT": wvT, "wpT": wpT})

    trace = TRACE and _maybe_install_trace_hook()
    res = run_bass_kernel_spmd(nc, in_maps, list(range(W)), trace=trace)
    LAST_RESULT["exec_time_ns"] = res.exec_time_ns

    return np.concatenate([res.results[c]["out"] for c in range(W)], axis=0)


# revision 27
# speedup vs baseline: 1.1055x; 1.1055x over previous
"""Causal self-attention (T=4096, C=2048, 16 heads) on 8 TRN2 NeuronCores.

Sharding: tensor-parallel over heads (2 heads/core) for QKV + attention,
then per-head AllToAlls redistribute the attention output to
token-parallel (512 tokens/core) for the output projection.

This version merges the QKV projection (phase 1) and attention (phase 2)
into one interleaved instruction stream: query-chunk j's score/exp/PV
pipeline runs while token-chunk j+1's QKV matmuls keep the tensor engine
dense, so the scalar engine's exp throughput (the phase-2 binding
resource) hides entirely under phase-1 matmuls and chunk-tail latency
never idles the PE. Softmax denominators are computed on the PE via a
ones-matmul (broadcast partition reduce); gpsimd runs ONLY the two
AllToAlls, so no chunk tail can queue behind an in-flight collective.
Attention-output DMA loads are issued after all a2a_in writes so an
in-flight collective never blocks the sync queue. Phase 3 splits
even/odd source tiles so the head-0 half overlaps the head-1 AllToAll.
"""
import sys
import types

sys.path.insert(0, "/opt/trn_rl_repo")

import ml_dtypes
import numpy as np

from concourse import bacc, tile
import concourse.mybir as mybir
from concourse.bass_utils import run_bass_kernel_spmd

F32 = mybir.dt.float32
BF16 = mybir.dt.bfloat16
FP16 = mybir.dt.float16
NP_BF16 = np.dtype(ml_dtypes.bfloat16)

T, C = 4096, 2048
H, D = 16, 128
W = 8                  # cores
HL = H // W            # heads per core (2)
CL = HL * D            # local attention-output columns (256)
KT = C // 128          # contraction tiles (16)
TC1 = 512              # token chunk (phase 1 == phase 2 chunking)
NC1 = T // TC1         # 8
TC2 = 512
NC2 = T // TC2         # 8
TL = T // W            # tokens per core for the projection (512)
SCALE = float(1.0 / np.sqrt(D))

TRACE = False          # test harness sets kernel.TRACE = True for profiling
LAST_RESULT = {}       # test harness reads exec_time_ns from here

_cache = {}


def _build():
    nc = bacc.Bacc("TRN2", target_bir_lowering=False, debug=False, num_devices=W)
    xT_d = nc.dram_tensor("xT", [C, T], BF16, kind="ExternalInput")
    wqkT_d = nc.dram_tensor("wqkT", [C, 2 * CL], BF16, kind="ExternalInput")
    wvT_d = nc.dram_tensor("wvT", [C, CL], BF16, kind="ExternalInput")
    wpT_d = nc.dram_tensor("wpT", [C, C], BF16, kind="ExternalInput")
    out_d = nc.dram_tensor("out", [TL, C], F32, kind="ExternalOutput")

    with tile.TileContext(nc) as tc:
        with tc.tile_pool(name="res", bufs=1) as res, \
             tc.tile_pool(name="dram", bufs=1, space="DRAM") as dram:
            # per-head A2A buffers (bf16, normalized): shard j = token chunk
            # j. One collective per head: a 512KB 4-way split measured the
            # first op at 9 GB/s (per-collective barrier/startup dominates).
            a2a_in = [dram.tile([W, 128, TC2], BF16, tag=f"a2a_in{h}",
                                name=f"a2a_in{h}") for h in range(HL)]
            a2a_out = [dram.tile([W, 128, TC2], BF16, tag=f"a2a_out{h}",
                                 name=f"a2a_out{h}") for h in range(HL)]

            # resident q/k (transposed, [d, t]) and V ([s, d]), fp16
            qT = [res.tile([128, T], FP16, tag=f"qT{h}", name=f"qT{h}")
                  for h in range(HL)]
            kT = [res.tile([128, T], FP16, tag=f"kT{h}", name=f"kT{h}")
                  for h in range(HL)]
            V = [res.tile([128, CL], FP16, tag=f"V{i}", name=f"V{i}")
                 for i in range(T // 128)]

            # all-ones [128, 128] fp16: lhsT for the denominator matmul
            # (ones.T @ esum broadcasts the partition sum to all partitions)
            ones32 = res.tile([128, 128], F32, tag="ones32", name="ones32")
            nc.gpsimd.memset(ones32[:], 1.0)
            ones = res.tile([128, 128], FP16, tag="ones", name="ones")
            nc.vector.tensor_copy(ones[:], ones32[:])

            # 4 diagonal causal masks (keep where t >= s within the tile):
            # mask dk applies to s-tile k = 4j + dk of query chunk j
            masks = []
            for dk in range(4):
                m32 = res.tile([128, TC2], F32, tag="m32", name=f"m32_{dk}")
                nc.gpsimd.memset(m32[:], 1.0)
                mb = res.tile([128, TC2], FP16, tag=f"mask{dk}",
                              name=f"mask{dk}")
                nc.vector.tensor_copy(mb[:], m32[:])
                nc.gpsimd.affine_select(
                    out=mb[:], in_=mb[:],
                    compare_op=mybir.AluOpType.is_ge,
                    fill=0.0,
                    base=-128 * dk,
                    channel_multiplier=-1,
                    pattern=[[1, TC2]],
                )
                masks.append(mb)

            # projection weight, resident for phase 3 (prefetched mid-stream)
            wp = [[None] * KT for _ in range(C // 512)]

            # ---------------- merged phases 1+2 ----------------
            with tc.tile_pool(name="wpool", bufs=1) as wpool, \
                 tc.tile_pool(name="xpool", bufs=2) as xpool, \
                 tc.tile_pool(name="ph2", bufs=4) as p2, \
                 tc.tile_pool(name="es", bufs=2) as es, \
                 tc.tile_pool(name="dn", bufs=2) as dn, \
                 tc.tile_pool(name="a2s", bufs=3) as a2s, \
                 tc.tile_pool(name="pqp", bufs=1, space="PSUM") as pqp, \
                 tc.tile_pool(name="pvp", bufs=1, space="PSUM") as pvp, \
                 tc.tile_pool(name="ps2s", bufs=2, space="PSUM") as ps2s, \
                 tc.tile_pool(name="ps2o", bufs=1, space="PSUM") as ps2o, \
                 tc.tile_pool(name="psd", bufs=1, space="PSUM") as psd:
                wqk = [[None] * 4 for _ in range(KT)]
                wv = [None] * KT

                # zero-init the 4 e buffers: the diagonal mask multiplies
                # read the full half-tile, and 0 x stale-NaN (uninitialized
                # SBUF) would poison esum on each buffer's first use
                for i_ in range(4):
                    et = p2.tile([128, 2 * TC2], FP16, tag="e",
                                 name=f"einit{i_}")
                    nc.gpsimd.memset(et[:], 0.0)

                def load_x(j):
                    ts = []
                    for k in range(KT):
                        t_ = xpool.tile([128, TC1], BF16, tag=f"x{k}",
                                        name=f"x{j}_{k}")
                        nc.sync.dma_start(
                            t_[:],
                            xT_d.ap()[k * 128:(k + 1) * 128,
                                      j * TC1:(j + 1) * TC1],
                        )
                        ts.append(t_)
                    return ts

                # initial DMA: x chunk 0 interleaved with wqk column 0 + wv
                # (everything the m=0 group needs), then wqk columns 1-3
                xt0 = []
                for k in range(KT):
                    t_ = xpool.tile([128, TC1], BF16, tag=f"x{k}",
                                    name=f"x0_{k}")
                    nc.sync.dma_start(
                        t_[:], xT_d.ap()[k * 128:(k + 1) * 128, 0:TC1])
                    xt0.append(t_)
                    t2 = wpool.tile([128, 128], BF16, tag=f"wqk{k}_0",
                                    name=f"wqk{k}_0")
                    nc.sync.dma_start(
                        t2[:], wqkT_d.ap()[k * 128:(k + 1) * 128, 0:128])
                    wqk[k][0] = t2
                    t3 = wpool.tile([128, CL], BF16, tag=f"wv{k}",
                                    name=f"wv{k}")
                    nc.sync.dma_start(
                        t3[:], wvT_d.ap()[k * 128:(k + 1) * 128, :])
                    wv[k] = t3
                for m in range(1, 4):
                    for k in range(KT):
                        t2 = wpool.tile([128, 128], BF16, tag=f"wqk{k}_{m}",
                                        name=f"wqk{k}_{m}")
                        nc.sync.dma_start(
                            t2[:],
                            wqkT_d.ap()[k * 128:(k + 1) * 128,
                                        m * 128:(m + 1) * 128])
                        wqk[k][m] = t2

                # ---- phase-2 pair pipeline over (head, chunk, pair) ----
                # head 1 lags head 0 by one chunk so that (h0,7) completes
                # ~34us of pair work before the stream ends: the first A2A's
                # barrier skew + transfer hides under h1's trailing chunks
                chunk_order = [(0, 0)]
                for j in range(1, NC2):
                    chunk_order += [(0, j), (1, j - 1)]
                chunk_order += [(1, NC2 - 1)]
                pair_seq = []
                for h, j in chunk_order:
                    for p in range(2 * (j + 1)):
                        pair_seq.append((h, j, p))
                NPAIRS = len(pair_seq)  # 144

                cstate = {}

                def chunk_state(h, j):
                    key = (h, j)
                    if key not in cstate:
                        cstate[key] = {
                            "po": ps2o.tile([128, TC2], F32, tag="po",
                                            name=f"po_{h}_{j}"),
                            "esum": es.tile([128, TC2], FP16, tag="esum",
                                            name=f"esum_{h}_{j}"),
                        }
                    return cstate[key]

                def diag_off(j, k):
                    # diagonal s-tile k only scores queries t >= 128*dk
                    dk = k - 4 * j
                    return 128 * dk if dk > 0 else 0

                def emit_pair_scores(h, j, p):
                    ps = ps2s.tile([128, 2 * TC2], F32, tag="ps",
                                   name=f"ps_{h}_{j}_{p}")
                    for half in range(2):
                        k = 2 * p + half
                        off = diag_off(j, k)
                        nc.tensor.matmul(
                            ps[:, half * TC2 + off:(half + 1) * TC2],
                            kT[h][:, k * 128:(k + 1) * 128],
                            qT[h][:, j * TC2 + off:(j + 1) * TC2],
                            start=True, stop=True)
                    return ps

                emitted = {}
                # cap: scores for pair idx >= cap may not be emitted yet —
                # their qT/kT source chunk is still being evacuated. Raised
                # per m-group as the evacs are emitted.
                cur = {"emit": 0, "cons": 0, "cap": 4}
                LA = 2

                def consume(idx):
                    h, j, p = pair_seq[idx]
                    nk = (j + 1) * 4
                    st = chunk_state(h, j)
                    ps = emitted.pop(idx)
                    e = p2.tile([128, 2 * TC2], FP16, tag="e", name=f"e{idx}")
                    if p >= 2 * j:
                        # diagonal pair: the score matmuls only wrote
                        # [off:] of each half — exp of the unwritten PSUM
                        # region is stale garbage (observed NaN on HW), so
                        # narrow the activation to the written range and
                        # zero the causally-dead columns explicitly (on DVE,
                        # NOT gpsimd: a gpsimd memset in the tail would
                        # queue behind the blocking A2A instruction)
                        for half in range(2):
                            k = 2 * p + half
                            off = diag_off(j, k)
                            if off > 0:
                                nc.vector.memset(
                                    e[:, half * TC2:half * TC2 + off], 0.0)
                            nc.scalar.activation(
                                e[:, half * TC2 + off:(half + 1) * TC2],
                                ps[:, half * TC2 + off:(half + 1) * TC2],
                                mybir.ActivationFunctionType.Exp,
                                scale=SCALE)
                    else:
                        # one exp per pair amortizes ACT init cost
                        nc.scalar.activation(
                            e[:], ps[:],
                            mybir.ActivationFunctionType.Exp,
                            scale=SCALE)
                    for half in range(2):
                        k = 2 * p + half
                        dk = k - 4 * j
                        if dk >= 0:
                            eh = e[:, half * TC2:(half + 1) * TC2]
                            nc.vector.tensor_mul(eh, eh, masks[dk][:])
                    for half in range(2):
                        k = 2 * p + half
                        eh = e[:, half * TC2:(half + 1) * TC2]
                        if k == 0:
                            nc.vector.tensor_copy(st["esum"][:], eh)
                        else:
                            nc.vector.tensor_add(st["esum"][:],
                                                 st["esum"][:], eh)
                    # emit the lookahead scores only AFTER this pair's exp is
                    # on the ACT queue: the new ps tile reuses the buffer of
                    # pair idx-LA+... the oldest live pair, and the WAR dep on
                    # its exp read only exists once that exp is emitted
                    ni = idx + LA
                    if ni < min(NPAIRS, cur["cap"]) and cur["emit"] <= ni:
                        emitted[ni] = emit_pair_scores(*pair_seq[ni])
                        cur["emit"] = ni + 1
                    for half in range(2):
                        # exp of a narrowed score tile leaves stale data left
                        # of `off`; the mask zeroed it for esum, and P@V
                        # skips those columns (causally zero for this s-tile)
                        k = 2 * p + half
                        off = diag_off(j, k)
                        nc.tensor.matmul(
                            st["po"][:, off:],
                            V[k][:, h * 128:(h + 1) * 128],
                            e[:, half * TC2 + off:(half + 1) * TC2],
                            start=(k == 0), stop=(k == nk - 1))
                    if 2 * p + 2 != nk:
                        return
                    # chunk tail: denominator via ones-matmul on the PE
                    # (broadcast partition reduce), reciprocal on DVE, then
                    # normalize po during PSUM evacuation
                    den = psd.tile([128, TC2], F32, tag="den",
                                   name=f"den_{h}_{j}")
                    nc.tensor.matmul(den[:], ones[:], st["esum"][:],
                                     start=True, stop=True)
                    # evacuate to SBUF first: reciprocal_approx_fast is a
                    # custom DVE op and reading PSUM directly produced NaNs
                    # on hardware
                    dsb = dn.tile([128, TC2], F32, tag="dsb",
                                  name=f"dsb_{h}_{j}")
                    nc.scalar.copy(dsb[:], den[:])
                    rec = dn.tile([128, TC2], F32, tag="rec",
                                  name=f"rec_{h}_{j}")
                    nc.vector.reciprocal_approx_fast(out=rec[:], in_=dsb[:])
                    att = a2s.tile([128, TC2], BF16, tag="att",
                                   name=f"att_{h}_{j}")
                    nc.vector.tensor_mul(att[:], st["po"][:], rec[:])
                    nc.sync.dma_start(a2a_in[h][j, :, :], att[:])
                    del cstate[(h, j)]
                    if j == NC2 - 1:
                        # fire this head's A2A the moment its data is ready;
                        # gpsimd carries nothing else, so the blocking
                        # collective instruction stalls no other work
                        nc.gpsimd.collective_compute(
                            "AllToAll",
                            mybir.AluOpType.bypass,
                            ins=[a2a_in[h].opt()],
                            outs=[a2a_out[h].opt()],
                            replica_groups=[list(range(W))],
                        )

                def pump(n):
                    for _ in range(n):
                        ci = cur["cons"]
                        if ci >= NPAIRS:
                            return
                        while cur["emit"] < min(ci + LA, NPAIRS,
                                                cur["cap"]):
                            ei = cur["emit"]
                            emitted[ei] = emit_pair_scores(*pair_seq[ei])
                            cur["emit"] += 1
                        assert cur["emit"] > ci, (ci, cur)
                        consume(ci)
                        cur["cons"] += 1

                def p1_group(xt, j, m, slots):
                    # one QKV m-group (32 MMs) split into 4 sub-blocks with
                    # phase-2 pairs pumped between them so ACT never lags
                    pq = pqp.tile([128, TC1], F32, tag="pq",
                                  name=f"pq{j}_{m}")
                    pv = pvp.tile([128, CL], F32, tag="pv", name=f"pv{j}_{m}")
                    for kb in range(0, KT, 4):
                        for k in range(kb, kb + 4):
                            nc.tensor.matmul(pq[:], wqk[k][m][:], xt[k][:],
                                             start=(k == 0),
                                             stop=(k == KT - 1))
                            nc.tensor.matmul(
                                pv[:],
                                xt[k][:, m * 128:(m + 1) * 128],
                                wv[k][:],
                                start=(k == 0), stop=(k == KT - 1))
                        pump(slots.pop(0) if slots else 0)
                    dest = qT[m] if m < HL else kT[m - HL]
                    nc.vector.tensor_copy(dest[:, j * TC1:(j + 1) * TC1],
                                          pq[:])
                    nc.scalar.copy(V[j * 4 + m][:], pv[:])

                def quota_slots(quota, nslots):
                    base = quota // nslots
                    slots = [base] * nslots
                    for z in range(quota - base * nslots):
                        slots[z] += 1
                    return slots

                # P1 chunk 0: no pairs ready yet
                for m in range(4):
                    p1_group(xt0, 0, m, [])
                # P1 chunks 1..7: pump the pairs of query chunk jj-1
                for jj in range(1, NC1):
                    xt = load_x(jj)
                    if jj == 2:
                        # prefetch the projection weight behind x chunk 2
                        for oc in range(C // 512):
                            for kc in range(KT):
                                t_ = res.tile([128, 512], BF16,
                                              tag=f"wp{oc}_{kc}",
                                              name=f"wp{oc}_{kc}")
                                nc.sync.dma_start(
                                    t_[:],
                                    wpT_d.ap()[kc * 128:(kc + 1) * 128,
                                               oc * 512:(oc + 1) * 512],
                                )
                                wp[oc][kc] = t_
                    # window jj consumes (h0, jj-1) + (h1, jj-2)
                    END = [2, 8, 18, 32, 50, 72, 112]
                    if jj < NC1 - 1:
                        slots = quota_slots(END[jj - 1] -
                                            (END[jj - 2] if jj > 1 else 0),
                                            17)
                        # cap: next-window (h0,jj) lookahead scores need qT0
                        # chunk jj — evacuated at the end of m=0's group
                        caps = [END[jj - 1]] + [NPAIRS] * 4
                    else:
                        # final window: (h0,6) + (h1,5) + h0c7 p0-13 (those
                        # need only qT0(7) [m=0 evac] and kT0/V <= chunk 6)
                        slots = quota_slots(40, 17)
                        # pair idx needs: 98..111 qT0(7) [m=0 evac],
                        # 112..113 kT0(7) [m=2], 128..141 qT1(7) [m=1],
                        # 142..143 kT1(7) [m=3]; caps[m] applies DURING
                        # group m, i.e. before that group's evac is emitted
                        caps = [98, 112, 112, 142, NPAIRS]
                    for m in range(4):
                        cur["cap"] = caps[m]
                        p1_group(xt, jj, m, slots[4 * m:4 * (m + 1)])
                    cur["cap"] = caps[4]
                    pump(sum(slots[16:]))
                # tail: (h0,7) p14-15 -> A2A h0 fires, then (h1,7)'s 16
                # pairs cover the h0 transfer, then A2A h1 (hidden under
                # phase-3's even half)
                pump(NPAIRS)

            # ---------------- phase 3: output projection ----------------
            with tc.tile_pool(name="p3a", bufs=1) as p3a, \
                 tc.tile_pool(name="acc3", bufs=1) as acc3, \
                 tc.tile_pool(name="p3o", bufs=3) as p3o, \
                 tc.tile_pool(name="ps3", bufs=2, space="PSUM") as ps3:
                attn = [None] * KT

                def load_attn(h):
                    # issued from the scalar (ACT) hwdge queue: idle at the
                    # tail, and a load waiting on an in-flight collective
                    # must not block att writes queued on sync behind it
                    for i in range(W):
                        kc = i * HL + h
                        t_ = p3a.tile([128, TL], BF16, tag=f"at{kc}",
                                      name=f"at{kc}")
                        nc.scalar.dma_start(t_[:], a2a_out[h][i, :, :])
                        attn[kc] = t_

                for h in range(HL):
                    load_attn(h)

                # even-kc halves (head-0 sourced, available before the
                # second A2A) run first into rotating psum banks with f32
                # SBUF spill, covering the head-1 A2A window; odd halves
                # then finish in psum and DVE adds the spill.
                accs = {}
                for oc in range(4):
                    for tt in range(TL // 128):
                        p3 = ps3.tile([128, 512], F32, tag="p3",
                                      name=f"p3e_{oc}_{tt}")
                        for kc in range(0, KT, 2):
                            nc.tensor.matmul(
                                p3[:],
                                attn[kc][:, tt * 128:(tt + 1) * 128],
                                wp[oc][kc][:],
                                start=(kc == 0), stop=(kc == KT - 2))
                        # spill on DVE: the scalar queue holds the attn load
                        # descriptors, which block on collective semaphores —
                        # a scalar copy behind them would stall the rotation
                        acc = acc3.tile([128, 512], F32,
                                        tag=f"acc{oc}_{tt}",
                                        name=f"acc{oc}_{tt}")
                        nc.vector.tensor_copy(acc[:], p3[:])
                        accs[(oc, tt)] = acc
                for oc in range(4):
                    for tt in range(TL // 128):
                        p3 = ps3.tile([128, 512], F32, tag="p3",
                                      name=f"p3o_{oc}_{tt}")
                        for kc in range(1, KT, 2):
                            nc.tensor.matmul(
                                p3[:],
                                attn[kc][:, tt * 128:(tt + 1) * 128],
                                wp[oc][kc][:],
                                start=(kc == 1), stop=(kc == KT - 1))
                        ob = p3o.tile([128, 512], F32, tag="ob")
                        nc.vector.tensor_add(ob[:], accs[(oc, tt)][:],
                                             p3[:])
                        nc.sync.dma_start(
                            out_d.ap()[tt * 128:(tt + 1) * 128,
                                       oc * 512:(oc + 1) * 512],
                            ob[:])

    nc.compile()
    return nc


def _maybe_install_trace_hook():
    try:
        import antenv
        from trn_agent_boot.trn_boot import _ntff_profile_via_ctypes
        hook = _ntff_profile_via_ctypes("/opt/axon/libaxon_pjrt.so")
        mod = types.ModuleType("antenv.axon_hooks")
        mod.get_axon_ntff_profile_hook = lambda: hook
        mod.set_axon_ntff_profile_hook = lambda h: None
        sys.modules["antenv.axon_hooks"] = mod
        antenv.axon_hooks = mod
        return True
    except Exception:
        return False


def kernel(x, w_attn, w_proj):
    x = np.ascontiguousarray(x, dtype=np.float32)
    w_attn = np.ascontiguousarray(w_attn, dtype=np.float32)
    w_proj = np.ascontiguousarray(w_proj, dtype=np.float32)

    if "nc" not in _cache:
        _cache["nc"] = _build()
    nc = _cache["nc"]

    xT = np.ascontiguousarray(x.T).astype(NP_BF16)
    wpT = np.ascontiguousarray(w_proj.T).astype(NP_BF16)
    in_maps = []
    for c in range(W):
        r0 = CL * c
        wqk = np.concatenate(
            [w_attn[r0:r0 + CL], w_attn[C + r0:C + r0 + CL]], axis=0)
        wqkT = np.ascontiguousarray(wqk.T).astype(NP_BF16)
        wvT = np.ascontiguousarray(
            w_attn[2 * C + r0:2 * C + r0 + CL].T).astype(NP_BF16)
        in_maps.append({"xT": xT, "wqkT": wqkT, "wvT": wvT, "wpT": wpT})

    trace = TRACE and _maybe_install_trace_hook()
    res = run_bass_kernel_spmd(nc, in_maps, list(range(W)), trace=trace)
    LAST_RESULT["exec_time_ns"] = res.exec_time_ns

    return np.concatenate([res.results[c]["out"] for c in range(W)], axis=0)
